# revision 23
# baseline (speedup 1.0000x reference)
"""Multi-head attention with RoPE on 8 Trainium2 NeuronCores.

Sharding: core c handles batch b = c//2 and head-group hg = c%2 (8 of 16
heads).  Data-parallel over batch, tensor-parallel over heads; the
row-parallel wo all-reduce (2 cores per batch) happens on the host during
the gather/unshard step.

v2 (all-bf16, pipelined):
  * All matmul operands bf16 (fp32 PSUM accumulate): 1 cycle/row at any N,
    FWL weight loads, half the DMA/SBUF traffic.  Host pre-casts.
  * RoPE fused into the Q/K projection epilogue per [128, 2048] row-tile:
    ACT/DVE copy PSUM->SBUF bf16, SBUF->SBUF DMA for the 32-row pair swap,
    3 DVE bf16 multiplies with precomputed cos/sin tables.
  * Attention is software-pipelined per (head-pair, q-chunk): the PE issues
    score(j) for both heads, then attnV(j-1), so the exp on ACT never
    blocks the PE.  Projections for the NEXT head-pair (and the wo
    projection during the last pair) are interleaved as PE filler work.
  * Softmax normalization: denominator via a ones-column in V (row 64 of
    the attnV accumulator), reciprocal_approx_fast on DVE, head-pair
    broadcast with one tiny f32r matmul, DVE multiply.
"""

import sys
import types

sys.path.insert(0, "/opt/trn_rl_repo")

import numpy as np
import ml_dtypes

import concourse.bacc as bacc
import concourse.mybir as mybir
import concourse.tile as tile
from concourse.bass_utils import run_bass_kernel_spmd

# Problem constants (hardcoded per contract)
B, S, D = 4, 2048, 1024
H = 16
DH = D // H          # 64
THETA = 10000.0
NCORES = 8
HG = 2               # head groups (tensor-parallel factor)
HD = D // HG         # 512 = per-core heads dim
NH = H // HG         # 8 heads per core
P = 128
SC = 512             # q-chunk
NSC = S // SC        # 4
NKB = S // P         # 16 k-blocks
NDB = D // P         # 8 d-blocks (contraction for projections)
NMT = HD // P        # 4 row-tiles of Q/K/A per core
SCALE = 1.0 / np.sqrt(np.float32(DH))

F32 = mybir.dt.float32
F32R = mybir.dt.float32r
BF16 = mybir.dt.bfloat16
NPBF16 = ml_dtypes.bfloat16


def _install_ntff_hook():
    """Best-effort: register the axon NTFF profile hook so trace=True works."""
    try:
        import antenv

        if "antenv.axon_hooks" in sys.modules:
            return
        sys.path.insert(0, "/root/.axon_site/trn_agent_boot")
        import trn_boot

        hook = trn_boot._ntff_profile_via_ctypes("/opt/axon/libaxon_pjrt.so")
        mod = types.ModuleType("antenv.axon_hooks")
        mod.get_axon_ntff_profile_hook = lambda: hook
        mod.set_axon_ntff_profile_hook = lambda h: None
        sys.modules["antenv.axon_hooks"] = mod
        antenv.axon_hooks = mod
    except Exception:
        pass


def build_program():
    nc = bacc.Bacc("TRN2", target_bir_lowering=False, debug=False,
                   num_devices=NCORES)

    xt_d = nc.dram_tensor("xt", [D, S], BF16, kind="ExternalInput")
    wqt_d = nc.dram_tensor("wqt", [D, HD], BF16, kind="ExternalInput")
    wkt_d = nc.dram_tensor("wkt", [D, HD], BF16, kind="ExternalInput")
    wvt_d = nc.dram_tensor("wvt", [D, HD], BF16, kind="ExternalInput")
    wot_d = nc.dram_tensor("wot", [HD, D], BF16, kind="ExternalInput")
    cf_d = nc.dram_tensor("cfull", [P, S], BF16, kind="ExternalInput")
    sf_d = nc.dram_tensor("sfull", [P, S], BF16, kind="ExternalInput")
    tri_d = nc.dram_tensor("tri", [P, P], BF16, kind="ExternalInput")
    ones_d = nc.dram_tensor("ones", [P, NH], BF16, kind="ExternalInput")
    eye2_d = nc.dram_tensor("eye2", [DH, P], F32R, kind="ExternalInput")
    zs_d = nc.dram_tensor("zs", [DH, SC], F32R, kind="ExternalInput")
    out_d = nc.dram_tensor("outT", [D, S], BF16, kind="ExternalOutput")

    EXP = mybir.ActivationFunctionType.Exp
    MULT = mybir.AluOpType.mult
    ADD = mybir.AluOpType.add

    with tile.TileContext(nc) as tc:
        with (
            tc.tile_pool(name="big", bufs=1) as big,      # persistent [P,S]
            tc.tile_pool(name="vg", bufs=1) as vgp,       # persistent V
            tc.tile_pool(name="w", bufs=24) as wp,        # wq+wk+wv resident
            tc.tile_pool(name="wo", bufs=4) as wop,
            tc.tile_pool(name="raw", bufs=2) as rawp,
            tc.tile_pool(name="sw", bufs=2) as swp,
            tc.tile_pool(name="at", bufs=5) as atp,
            tc.tile_pool(name="rr", bufs=2) as rrp,
            tc.tile_pool(name="rbc", bufs=2) as rbp,
            tc.tile_pool(name="wout", bufs=4) as woutp,
            tc.tile_pool(name="small", bufs=1) as small,
            tc.tile_pool(name="psS", bufs=2, space="PSUM") as psS,
            tc.tile_pool(name="psV", bufs=2, space="PSUM") as psV,
            tc.tile_pool(name="psA", bufs=2, space="PSUM") as psA,
        ):
            # ---- input loads, spread across engine queues so the issue
            # serialization doesn't delay the first projection matmuls ----
            xt = []
            for k in range(NDB):
                t = big.tile([P, S], BF16, tag=f"xt{k}", name=f"xt{k}")
                nc.gpsimd.dma_start(t[:], xt_d[P * k:P * (k + 1), :])
                xt.append(t)

            def load_w(dram, nm, eng):
                ts_ = []
                for k in range(NDB):
                    t = wp.tile([P, HD], BF16, tag="w", name=f"{nm}{k}")
                    eng.dma_start(t[:], dram[P * k:P * (k + 1), :])
                    ts_.append(t)
                return ts_

            wq_t = load_w(wqt_d, "wq", nc.sync)
            wk_t = load_w(wkt_d, "wk", nc.sync)
            wv_t = load_w(wvt_d, "wv", nc.gpsimd)

            tri = small.tile([P, P], BF16, tag="tri", name="tri")
            nc.scalar.dma_start(tri[:], tri_d[:])
            eye2 = small.tile([DH, P], F32R, tag="eye2", name="eye2")
            nc.scalar.dma_start(eye2[:], eye2_d[:])
            # persistent denominator staging: rows 0/32 live, others zero
            dn2 = small.tile([DH, SC], F32R, tag="dn2", name="dn2")
            nc.scalar.dma_start(dn2[:], zs_d[:])
            ones = small.tile([P, NH], BF16, tag="ones", name="ones")
            nc.scalar.dma_start(ones[:], ones_d[:])
            cf = big.tile([P, S], BF16, tag="cf", name="cf")
            nc.scalar.dma_start(cf[:], cf_d[:])
            sf = big.tile([P, S], BF16, tag="sf", name="sf")
            nc.scalar.dma_start(sf[:], sf_d[:])

            wo_t = []
            for k in range(NMT):
                t = wop.tile([P, D], BF16, tag="wo", name=f"wo{k}")
                nc.scalar.dma_start(t[:], wot_d[P * k:P * (k + 1), :])
                wo_t.append(t)

            # ---- persistent result tiles ----
            QT = [big.tile([P, S], BF16, tag=f"qt{m}", name=f"qt{m}")
                  for m in range(NMT)]
            KT = [big.tile([P, S], BF16, tag=f"kt{m}", name=f"kt{m}")
                  for m in range(NMT)]
            A = [big.tile([P, S], BF16, tag=f"a{m}", name=f"a{m}")
                 for m in range(NMT)]
            Vg = [vgp.tile([P, NH * (DH + 1)], BF16, tag=f"vg{j}",
                           name=f"vg{j}") for j in range(NKB)]

            # ---- emission helpers ----
            def proj_group(wt, m, n, raw, use_act):
                """One [128,512] projection chunk: 8 accum MMs + copy."""
                ps = psA.tile([P, SC], F32, tag="p", name="pp")
                for k in range(NDB):
                    nc.tensor.matmul(
                        ps[:],
                        wt[k][:, P * m:P * (m + 1)],
                        xt[k][:, SC * n:SC * (n + 1)],
                        start=(k == 0), stop=(k == NDB - 1),
                    )
                if use_act:
                    nc.scalar.copy(raw[:, SC * n:SC * (n + 1)], ps[:])
                else:
                    nc.vector.tensor_copy(raw[:, SC * n:SC * (n + 1)], ps[:])

            def rope(raw, dst):
                """dst = raw*cos + pairswap(raw)*sin over [128, 2048]."""
                sw = swp.tile([P, S], BF16, tag="sw", name="sw")
                for g in range(4):
                    src = (g ^ 1) * 32
                    nc.sync.dma_start(sw[g * 32:(g + 1) * 32, :],
                                      raw[src:src + 32, :])
                nc.vector.tensor_tensor(dst[:], raw[:], cf[:], MULT)
                nc.vector.tensor_tensor(sw[:], sw[:], sf[:], MULT)
                nc.vector.tensor_tensor(dst[:], dst[:], sw[:], ADD)

            def qk_tile_events(m, use_act):
                """Filler closures projecting+roping QT[m] and KT[m]."""
                evs = []
                for wt, dst, nm in ((wq_t, QT[m], "q"), (wk_t, KT[m], "k")):
                    raw = rawp.tile([P, S], BF16, tag="raw",
                                    name=f"raw{nm}{m}")
                    for n in range(NSC):
                        evs.append(lambda wt=wt, n=n, raw=raw:
                                   proj_group(wt, m, n, raw, use_act))
                    evs.append(lambda raw=raw, dst=dst: rope(raw, dst))
                return evs

            def wo_group(m8, n):
                ps = psA.tile([P, SC], F32, tag="p", name="pw")
                for k in range(NMT):
                    nc.tensor.matmul(
                        ps[:],
                        wo_t[k][:, P * m8:P * (m8 + 1)],
                        A[k][:, SC * n:SC * (n + 1)],
                        start=(k == 0), stop=(k == NMT - 1),
                    )
                ot = woutp.tile([P, SC], BF16, tag="wout", name="wout")
                nc.scalar.copy(ot[:], ps[:])
                nc.sync.dma_start(
                    out_d[P * m8:P * (m8 + 1), SC * n:SC * (n + 1)], ot[:])

            # ---- phase 1: Q[0], K[0] projections + rope (ACT copies) ----
            for ev in qk_tile_events(0, use_act=True):
                ev()

            # ---- phase 2: V projection ----
            for j in range(NKB):
                v3 = Vg[j][:].rearrange("p (h c) -> p h c", h=NH)
                nc.sync.dma_start(v3[:, :, DH:DH + 1], ones[:, :, None])
                ps = psA.tile([P, SC], F32, tag="p", name="pv")
                for k in range(NDB):
                    nc.tensor.matmul(
                        ps[:],
                        xt[k][:, P * j:P * (j + 1)],
                        wv_t[k][:],
                        start=(k == 0), stop=(k == NDB - 1),
                    )
                nc.vector.tensor_copy(
                    v3[:, :, 0:DH], ps[:].rearrange("p (h c) -> p h c", h=NH))

            # ---- phase 3: attention per head pair, pipelined ----
            def epilogue(av_a, av_b, qc, m):
                with nc.allow_low_precision(reason="f32r matmul feed"):
                    nc.vector.tensor_copy(dn2[0:1, :],
                                          av_a[DH:DH + 1, :])
                    nc.vector.tensor_copy(dn2[32:33, :],
                                          av_b[DH:DH + 1, :])
                bc = psA.tile([P, SC], F32, tag="p", name="bc")
                nc.tensor.matmul(bc[:], eye2[:], dn2[:],
                                 start=True, stop=True)
                rbc = rbp.tile([P, SC], F32, tag="rbc", name="rbc")
                nc.vector.reciprocal_approx_fast(rbc[:], bc[:])
                nc.vector.tensor_tensor(A[m][0:DH, SC * qc:SC * (qc + 1)],
                                        av_a[0:DH, :], rbc[0:DH, :], MULT)
                nc.vector.tensor_tensor(A[m][DH:P, SC * qc:SC * (qc + 1)],
                                        av_b[0:DH, :], rbc[DH:P, :], MULT)

            for hp in range(NMT):
                m = hp
                ha, hb = 2 * hp, 2 * hp + 1
                fillers = qk_tile_events(hp + 1, use_act=False) \
                    if hp + 1 < NMT else []
                pend = None

                def pop_filler():
                    if fillers:
                        fillers.pop(0)()

                for qc in range(NSC):
                    nj = 4 * qc + 4
                    av_a = psV.tile([DH + 1, SC], F32, tag="v",
                                    name=f"ava{hp}{qc}")
                    av_b = psV.tile([DH + 1, SC], F32, tag="v",
                                    name=f"avb{hp}{qc}")
                    ats = {}

                    def emit_avs(j):
                        d = j - 4 * qc
                        q0 = P * d if d >= 0 else 0
                        at = ats.pop(j)
                        for av, h, off in ((av_a, ha, 0), (av_b, hb, SC)):
                            nc.tensor.matmul(
                                av[:, q0:SC],
                                Vg[j][:, (DH + 1) * h:(DH + 1) * (h + 1)],
                                at[:, off + q0:off + SC],
                                start=(j == 0), stop=(j == nj - 1),
                            )

                    for j in range(nj):
                        d = j - 4 * qc
                        q0 = P * d if d >= 0 else 0
                        # both heads' score blocks into one 2-bank tile
                        ps = psS.tile([P, 2 * SC], F32, tag="s", name="ps")
                        at = atp.tile([P, 2 * SC], BF16, tag="at", name="at")
                        for r0, off in ((0, 0), (DH, SC)):
                            nc.tensor.matmul(
                                ps[:, off + q0:off + SC],
                                KT[m][r0:r0 + DH, P * j:P * (j + 1)],
                                QT[m][r0:r0 + DH, SC * qc + q0:SC * (qc + 1)],
                                start=True, stop=True,
                            )
                        if q0 == 0:
                            # one fused exp over both heads' halves
                            nc.scalar.activation(at[:], ps[:], EXP,
                                                 scale=float(SCALE))
                        else:
                            nc.scalar.activation(at[:, q0:SC], ps[:, q0:SC],
                                                 EXP, scale=float(SCALE))
                            nc.scalar.activation(at[:, SC + q0:2 * SC],
                                                 ps[:, SC + q0:2 * SC],
                                                 EXP, scale=float(SCALE))
                        if d >= 0:
                            for off in (0, SC):
                                nc.vector.tensor_tensor(
                                    at[:, off + q0:off + q0 + P],
                                    at[:, off + q0:off + q0 + P],
                                    tri[:], MULT)
                        ats[j] = at
                        if j == 2 and pend is not None:
                            epilogue(*pend)
                            if hp == NMT - 1:
                                for m8 in range(NDB):
                                    fillers.append(
                                        lambda m8=m8, n=pend[2]:
                                        wo_group(m8, n))
                            pend = None
                        if j >= 2:
                            emit_avs(j - 2)
                        if hp == NMT - 1 or j % 4 == 2:
                            pop_filler()
                    emit_avs(nj - 2)
                    emit_avs(nj - 1)
                    pend = (av_a, av_b, qc, m)
                    pop_filler()
                # last q-chunk epilogue of this head pair
                epilogue(*pend)
                if hp == NMT - 1:
                    for m8 in range(NDB):
                        fillers.append(
                            lambda m8=m8, n=pend[2]: wo_group(m8, n))
                while fillers:
                    fillers.pop(0)()

    nc.compile()
    return nc


_NC_CACHE = []


def _get_nc():
    if not _NC_CACHE:
        _NC_CACHE.append(build_program())
    return _NC_CACHE[0]


def _host_tables(token_positions):
    pos = np.asarray(token_positions).astype(np.float32)
    inv_freq = np.float32(THETA) ** (
        -np.arange(0, DH, 2, dtype=np.float32) / np.float32(DH))
    ang = pos[:, None] * inv_freq[None, :]            # [S, 32] f32
    cos_t = np.ascontiguousarray(np.cos(ang).T)        # [32, S]
    sin_t = np.ascontiguousarray(np.sin(ang).T)
    cfull = np.tile(cos_t, (4, 1)).astype(np.float32)  # [128, S]
    sfull = np.concatenate([-sin_t, sin_t, -sin_t, sin_t], 0).astype(np.float32)
    return cfull, sfull


def prepare_in_maps(inputs):
    """Build the per-core input dicts (host-side shard + bf16 cast)."""
    x = np.asarray(inputs["in_features"], dtype=np.float32)
    wq = np.asarray(inputs["wq"], dtype=np.float32)
    wk = np.asarray(inputs["wk"], dtype=np.float32)
    wv = np.asarray(inputs["wv"], dtype=np.float32)
    wo = np.asarray(inputs["wo"], dtype=np.float32)

    cfull, sfull = _host_tables(inputs["token_positions"])
    tri = np.triu(np.ones((P, P), dtype=np.float32)).astype(NPBF16)
    ones = np.ones((P, NH), dtype=NPBF16)
    eye2 = np.zeros((DH, P), dtype=np.float32)
    eye2[0, :DH] = 1.0
    eye2[32, DH:] = 1.0
    zs = np.zeros((DH, SC), dtype=np.float32)

    # per-head row permutation: evens then odds
    perm1 = np.concatenate([np.arange(0, DH, 2), np.arange(1, DH, 2)])
    perm = np.concatenate([h * DH + perm1 for h in range(NH)])

    in_maps = []
    for c in range(NCORES):
        b, hg = divmod(c, HG)
        sl = slice(hg * HD, (hg + 1) * HD)
        in_maps.append({
            "xt": np.ascontiguousarray(x[b].T).astype(NPBF16),
            "wqt": np.ascontiguousarray(wq[sl][perm].T).astype(NPBF16),
            "wkt": np.ascontiguousarray(wk[sl][perm].T).astype(NPBF16),
            "wvt": np.ascontiguousarray(wv[sl].T).astype(NPBF16),
            "wot": np.ascontiguousarray(wo[:, sl].T).astype(NPBF16),
            "cfull": cfull.astype(NPBF16),
            "sfull": sfull.astype(NPBF16),
            "tri": tri,
            "ones": ones,
            "eye2": eye2,
            "zs": zs,
        })
    return in_maps


def kernel(in_features, token_positions, wq, wk, wv, wo):
    _install_ntff_hook()
    in_maps = prepare_in_maps(dict(
        in_features=in_features, token_positions=token_positions,
        wq=wq, wk=wk, wv=wv, wo=wo))

    nc = _get_nc()
    res = run_bass_kernel_spmd(nc, in_maps, list(range(NCORES)))

    out = np.empty((B, S, D), dtype=np.float32)
    for b in range(B):
        acc = (np.asarray(res.results[2 * b]["outT"], dtype=np.float32)
               + np.asarray(res.results[2 * b + 1]["outT"], dtype=np.float32))
        out[b] = acc.T
    return out


# revision 24
# speedup vs baseline: 1.1278x; 1.1278x over previous
"""Multi-head attention with RoPE on 8 Trainium2 NeuronCores.

Sharding: core c handles batch b = c//2 and head-group hg = c%2 (8 of 16
heads).  Data-parallel over batch, tensor-parallel over heads; the
row-parallel wo all-reduce (2 cores per batch) happens on the host during
the gather/unshard step.

v2 (all-bf16, pipelined):
  * All matmul operands bf16 (fp32 PSUM accumulate): 1 cycle/row at any N,
    FWL weight loads, half the DMA/SBUF traffic.  Host pre-casts.
  * RoPE fused into the Q/K projection epilogue per [128, 2048] row-tile:
    ACT/DVE copy PSUM->SBUF bf16, SBUF->SBUF DMA for the 32-row pair swap,
    3 DVE bf16 multiplies with precomputed cos/sin tables.
  * Attention is software-pipelined per (head-pair, q-chunk): the PE issues
    score(j) for both heads, then attnV(j-1), so the exp on ACT never
    blocks the PE.  Projections for the NEXT head-pair (and the wo
    projection during the last pair) are interleaved as PE filler work.
  * Softmax normalization: denominator via a ones-column in V (row 64 of
    the attnV accumulator), reciprocal_approx_fast on DVE, head-pair
    broadcast with one tiny f32r matmul, DVE multiply.
"""

import sys
import types

sys.path.insert(0, "/opt/trn_rl_repo")

import numpy as np
import ml_dtypes

import concourse.bacc as bacc
import concourse.mybir as mybir
import concourse.tile as tile
from concourse.bass_utils import run_bass_kernel_spmd

# Problem constants (hardcoded per contract)
B, S, D = 4, 2048, 1024
H = 16
DH = D // H          # 64
THETA = 10000.0
NCORES = 8
HG = 2               # head groups (tensor-parallel factor)
HD = D // HG         # 512 = per-core heads dim
NH = H // HG         # 8 heads per core
P = 128
SC = 512             # q-chunk
NSC = S // SC        # 4
NKB = S // P         # 16 k-blocks
NDB = D // P         # 8 d-blocks (contraction for projections)
NMT = HD // P        # 4 row-tiles of Q/K/A per core
SCALE = 1.0 / np.sqrt(np.float32(DH))

F32 = mybir.dt.float32
F32R = mybir.dt.float32r
BF16 = mybir.dt.bfloat16
NPBF16 = ml_dtypes.bfloat16


def _install_ntff_hook():
    """Best-effort: register the axon NTFF profile hook so trace=True works."""
    try:
        import antenv

        if "antenv.axon_hooks" in sys.modules:
            return
        sys.path.insert(0, "/root/.axon_site/trn_agent_boot")
        import trn_boot

        hook = trn_boot._ntff_profile_via_ctypes("/opt/axon/libaxon_pjrt.so")
        mod = types.ModuleType("antenv.axon_hooks")
        mod.get_axon_ntff_profile_hook = lambda: hook
        mod.set_axon_ntff_profile_hook = lambda h: None
        sys.modules["antenv.axon_hooks"] = mod
        antenv.axon_hooks = mod
    except Exception:
        pass


def build_program():
    nc = bacc.Bacc("TRN2", target_bir_lowering=False, debug=False,
                   num_devices=NCORES)

    xt_d = nc.dram_tensor("xt", [D, S], BF16, kind="ExternalInput")
    wqt_d = nc.dram_tensor("wqt", [D, HD], BF16, kind="ExternalInput")
    wkt_d = nc.dram_tensor("wkt", [D, HD], BF16, kind="ExternalInput")
    wvt_d = nc.dram_tensor("wvt", [D, HD], BF16, kind="ExternalInput")
    wot_d = nc.dram_tensor("wot", [HD, D], BF16, kind="ExternalInput")
    cf_d = nc.dram_tensor("cfull", [P, S], BF16, kind="ExternalInput")
    sf_d = nc.dram_tensor("sfull", [P, S], BF16, kind="ExternalInput")
    tri_d = nc.dram_tensor("tri", [P, P], BF16, kind="ExternalInput")
    ones_d = nc.dram_tensor("ones", [P, NH], BF16, kind="ExternalInput")
    eye2_d = nc.dram_tensor("eye2", [DH, P], F32R, kind="ExternalInput")
    zs_d = nc.dram_tensor("zs", [DH, SC], F32R, kind="ExternalInput")
    out_d = nc.dram_tensor("outT", [D, S], BF16, kind="ExternalOutput")

    EXP = mybir.ActivationFunctionType.Exp
    MULT = mybir.AluOpType.mult
    ADD = mybir.AluOpType.add

    with tile.TileContext(nc) as tc:
        with (
            tc.tile_pool(name="big", bufs=1) as big,      # persistent [P,S]
            tc.tile_pool(name="vg", bufs=1) as vgp,       # persistent V
            tc.tile_pool(name="w", bufs=24) as wp,        # wq+wk+wv resident
            tc.tile_pool(name="wo", bufs=4) as wop,
            tc.tile_pool(name="raw", bufs=2) as rawp,
            tc.tile_pool(name="sw", bufs=2) as swp,
            tc.tile_pool(name="at", bufs=5) as atp,
            tc.tile_pool(name="rr", bufs=2) as rrp,
            tc.tile_pool(name="rbc", bufs=2) as rbp,
            tc.tile_pool(name="wout", bufs=4) as woutp,
            tc.tile_pool(name="small", bufs=1) as small,
            tc.tile_pool(name="psS", bufs=2, space="PSUM") as psS,
            tc.tile_pool(name="psV", bufs=2, space="PSUM") as psV,
            tc.tile_pool(name="psA", bufs=2, space="PSUM") as psA,
        ):
            # ---- input loads, spread across engine queues so the issue
            # serialization doesn't delay the first projection matmuls ----
            xt = []
            for k in range(NDB):
                t = big.tile([P, S], BF16, tag=f"xt{k}", name=f"xt{k}")
                nc.gpsimd.dma_start(t[:], xt_d[P * k:P * (k + 1), :])
                xt.append(t)

            def load_w(dram, nm, eng):
                ts_ = []
                for k in range(NDB):
                    t = wp.tile([P, HD], BF16, tag="w", name=f"{nm}{k}")
                    eng.dma_start(t[:], dram[P * k:P * (k + 1), :])
                    ts_.append(t)
                return ts_

            wq_t = load_w(wqt_d, "wq", nc.sync)
            wk_t = load_w(wkt_d, "wk", nc.sync)
            wv_t = load_w(wvt_d, "wv", nc.gpsimd)

            tri = small.tile([P, P], BF16, tag="tri", name="tri")
            nc.scalar.dma_start(tri[:], tri_d[:])
            eye2 = small.tile([DH, P], F32R, tag="eye2", name="eye2")
            nc.scalar.dma_start(eye2[:], eye2_d[:])
            # persistent denominator staging: rows 0/32 live, others zero
            dn2 = small.tile([DH, SC], F32R, tag="dn2", name="dn2")
            nc.scalar.dma_start(dn2[:], zs_d[:])
            ones = small.tile([P, NH], BF16, tag="ones", name="ones")
            nc.scalar.dma_start(ones[:], ones_d[:])
            cf = big.tile([P, S], BF16, tag="cf", name="cf")
            nc.scalar.dma_start(cf[:], cf_d[:])
            sf = big.tile([P, S], BF16, tag="sf", name="sf")
            nc.scalar.dma_start(sf[:], sf_d[:])

            wo_t = []
            for k in range(NMT):
                t = wop.tile([P, D], BF16, tag="wo", name=f"wo{k}")
                nc.scalar.dma_start(t[:], wot_d[P * k:P * (k + 1), :])
                wo_t.append(t)

            # ---- persistent result tiles ----
            QT = [big.tile([P, S], BF16, tag=f"qt{m}", name=f"qt{m}")
                  for m in range(NMT)]
            KT = [big.tile([P, S], BF16, tag=f"kt{m}", name=f"kt{m}")
                  for m in range(NMT)]
            A = [big.tile([P, S], BF16, tag=f"a{m}", name=f"a{m}")
                 for m in range(NMT)]
            Vg = [vgp.tile([P, NH * (DH + 1)], BF16, tag=f"vg{j}",
                           name=f"vg{j}") for j in range(NKB)]

            # ---- emission helpers ----
            def proj_group(wt, m, n, raw, use_act):
                """One [128,512] projection chunk: 8 accum MMs + copy."""
                ps = psA.tile([P, SC], F32, tag="p", name="pp")
                for k in range(NDB):
                    nc.tensor.matmul(
                        ps[:],
                        wt[k][:, P * m:P * (m + 1)],
                        xt[k][:, SC * n:SC * (n + 1)],
                        start=(k == 0), stop=(k == NDB - 1),
                    )
                if use_act:
                    nc.scalar.copy(raw[:, SC * n:SC * (n + 1)], ps[:])
                else:
                    nc.vector.tensor_copy(raw[:, SC * n:SC * (n + 1)], ps[:])

            def rope(raw, dst):
                """dst = raw*cos + pairswap(raw)*sin over [128, 2048]."""
                sw = swp.tile([P, S], BF16, tag="sw", name="sw")
                for g in range(4):
                    src = (g ^ 1) * 32
                    nc.sync.dma_start(sw[g * 32:(g + 1) * 32, :],
                                      raw[src:src + 32, :])
                nc.vector.tensor_tensor(dst[:], raw[:], cf[:], MULT)
                nc.vector.tensor_tensor(sw[:], sw[:], sf[:], MULT)
                nc.vector.tensor_tensor(dst[:], dst[:], sw[:], ADD)

            def qk_tile_events(m, use_act):
                """Filler closures projecting+roping QT[m] and KT[m]."""
                evs = []
                for wt, dst, nm in ((wq_t, QT[m], "q"), (wk_t, KT[m], "k")):
                    raw = rawp.tile([P, S], BF16, tag="raw",
                                    name=f"raw{nm}{m}")
                    for n in range(NSC):
                        evs.append(lambda wt=wt, n=n, raw=raw:
                                   proj_group(wt, m, n, raw, use_act))
                    evs.append(lambda raw=raw, dst=dst: rope(raw, dst))
                return evs

            def wo_group(m8, n):
                ps = psA.tile([P, SC], F32, tag="p", name="pw")
                for k in range(NMT):
                    nc.tensor.matmul(
                        ps[:],
                        wo_t[k][:, P * m8:P * (m8 + 1)],
                        A[k][:, SC * n:SC * (n + 1)],
                        start=(k == 0), stop=(k == NMT - 1),
                    )
                ot = woutp.tile([P, SC], BF16, tag="wout", name="wout")
                nc.scalar.copy(ot[:], ps[:])
                nc.sync.dma_start(
                    out_d[P * m8:P * (m8 + 1), SC * n:SC * (n + 1)], ot[:])

            # ---- phase 1: Q[0], K[0] projections + rope (ACT copies) ----
            for ev in qk_tile_events(0, use_act=True):
                ev()

            # ---- phase 2: V projection ----
            for j in range(NKB):
                v3 = Vg[j][:].rearrange("p (h c) -> p h c", h=NH)
                nc.sync.dma_start(v3[:, :, DH:DH + 1], ones[:, :, None])
                ps = psA.tile([P, SC], F32, tag="p", name="pv")
                for k in range(NDB):
                    nc.tensor.matmul(
                        ps[:],
                        xt[k][:, P * j:P * (j + 1)],
                        wv_t[k][:],
                        start=(k == 0), stop=(k == NDB - 1),
                    )
                nc.vector.tensor_copy(
                    v3[:, :, 0:DH], ps[:].rearrange("p (h c) -> p h c", h=NH))

            # ---- phase 3: attention per head pair, pipelined ----
            def epilogue(av_a, av_b, qc, m):
                with nc.allow_low_precision(reason="f32r matmul feed"):
                    nc.vector.tensor_copy(dn2[0:1, :],
                                          av_a[DH:DH + 1, :])
                    nc.vector.tensor_copy(dn2[32:33, :],
                                          av_b[DH:DH + 1, :])
                bc = psA.tile([P, SC], F32, tag="p", name="bc")
                nc.tensor.matmul(bc[:], eye2[:], dn2[:],
                                 start=True, stop=True)
                rbc = rbp.tile([P, SC], F32, tag="rbc", name="rbc")
                nc.vector.reciprocal_approx_fast(rbc[:], bc[:])
                nc.vector.tensor_tensor(A[m][0:DH, SC * qc:SC * (qc + 1)],
                                        av_a[0:DH, :], rbc[0:DH, :], MULT)
                nc.vector.tensor_tensor(A[m][DH:P, SC * qc:SC * (qc + 1)],
                                        av_b[0:DH, :], rbc[DH:P, :], MULT)

            for hp in range(NMT):
                m = hp
                ha, hb = 2 * hp, 2 * hp + 1
                fillers = qk_tile_events(hp + 1, use_act=False) \
                    if hp + 1 < NMT else []
                pend = None

                def pop_filler():
                    if fillers:
                        fillers.pop(0)()

                for qc in range(NSC):
                    nj = 4 * qc + 4
                    av_a = psV.tile([DH + 1, SC], F32, tag="v",
                                    name=f"ava{hp}{qc}")
                    av_b = psV.tile([DH + 1, SC], F32, tag="v",
                                    name=f"avb{hp}{qc}")
                    ats = {}

                    def emit_avs(j):
                        d = j - 4 * qc
                        q0 = P * d if d >= 0 else 0
                        at = ats.pop(j)
                        for av, h, off in ((av_a, ha, 0), (av_b, hb, SC)):
                            nc.tensor.matmul(
                                av[:, q0:SC],
                                Vg[j][:, (DH + 1) * h:(DH + 1) * (h + 1)],
                                at[:, off + q0:off + SC],
                                start=(j == 0), stop=(j == nj - 1),
                            )

                    for j in range(nj):
                        d = j - 4 * qc
                        q0 = P * d if d >= 0 else 0
                        # both heads' score blocks into one 2-bank tile
                        ps = psS.tile([P, 2 * SC], F32, tag="s", name="ps")
                        at = atp.tile([P, 2 * SC], BF16, tag="at", name="at")
                        for r0, off in ((0, 0), (DH, SC)):
                            nc.tensor.matmul(
                                ps[:, off + q0:off + SC],
                                KT[m][r0:r0 + DH, P * j:P * (j + 1)],
                                QT[m][r0:r0 + DH, SC * qc + q0:SC * (qc + 1)],
                                start=True, stop=True,
                            )
                        if q0 == 0:
                            # one fused exp over both heads' halves
                            nc.scalar.activation(at[:], ps[:], EXP,
                                                 scale=float(SCALE))
                        else:
                            nc.scalar.activation(at[:, q0:SC], ps[:, q0:SC],
                                                 EXP, scale=float(SCALE))
                            nc.scalar.activation(at[:, SC + q0:2 * SC],
                                                 ps[:, SC + q0:2 * SC],
                                                 EXP, scale=float(SCALE))
                        if d >= 0:
                            for off in (0, SC):
                                nc.gpsimd.tensor_tensor(
                                    at[:, off + q0:off + q0 + P],
                                    at[:, off + q0:off + q0 + P],
                                    tri[:], MULT)
                        ats[j] = at
                        if j == 2 and pend is not None:
                            epilogue(*pend)
                            if hp == NMT - 1:
                                for m8 in range(NDB):
                                    fillers.append(
                                        lambda m8=m8, n=pend[2]:
                                        wo_group(m8, n))
                            pend = None
                        if j >= 2:
                            emit_avs(j - 2)
                        if hp == NMT - 1 or j % 4 == 2:
                            pop_filler()
                    emit_avs(nj - 2)
                    emit_avs(nj - 1)
                    pend = (av_a, av_b, qc, m)
                    pop_filler()
                # last q-chunk epilogue of this head pair
                epilogue(*pend)
                if hp == NMT - 1:
                    for m8 in range(NDB):
                        fillers.append(
                            lambda m8=m8, n=pend[2]: wo_group(m8, n))
                while fillers:
                    fillers.pop(0)()

    nc.compile()
    return nc


_NC_CACHE = []


def _get_nc():
    if not _NC_CACHE:
        _NC_CACHE.append(build_program())
    return _NC_CACHE[0]


def _host_tables(token_positions):
    pos = np.asarray(token_positions).astype(np.float32)
    inv_freq = np.float32(THETA) ** (
        -np.arange(0, DH, 2, dtype=np.float32) / np.float32(DH))
    ang = pos[:, None] * inv_freq[None, :]            # [S, 32] f32
    cos_t = np.ascontiguousarray(np.cos(ang).T)        # [32, S]
    sin_t = np.ascontiguousarray(np.sin(ang).T)
    cfull = np.tile(cos_t, (4, 1)).astype(np.float32)  # [128, S]
    sfull = np.concatenate([-sin_t, sin_t, -sin_t, sin_t], 0).astype(np.float32)
    return cfull, sfull


def prepare_in_maps(inputs):
    """Build the per-core input dicts (host-side shard + bf16 cast)."""
    x = np.asarray(inputs["in_features"], dtype=np.float32)
    wq = np.asarray(inputs["wq"], dtype=np.float32)
    wk = np.asarray(inputs["wk"], dtype=np.float32)
    wv = np.asarray(inputs["wv"], dtype=np.float32)
    wo = np.asarray(inputs["wo"], dtype=np.float32)

    cfull, sfull = _host_tables(inputs["token_positions"])
    tri = np.triu(np.ones((P, P), dtype=np.float32)).astype(NPBF16)
    ones = np.ones((P, NH), dtype=NPBF16)
    eye2 = np.zeros((DH, P), dtype=np.float32)
    eye2[0, :DH] = 1.0
    eye2[32, DH:] = 1.0
    zs = np.zeros((DH, SC), dtype=np.float32)

    # per-head row permutation: evens then odds
    perm1 = np.concatenate([np.arange(0, DH, 2), np.arange(1, DH, 2)])
    perm = np.concatenate([h * DH + perm1 for h in range(NH)])

    in_maps = []
    for c in range(NCORES):
        b, hg = divmod(c, HG)
        sl = slice(hg * HD, (hg + 1) * HD)
        in_maps.append({
            "xt": np.ascontiguousarray(x[b].T).astype(NPBF16),
            "wqt": np.ascontiguousarray(wq[sl][perm].T).astype(NPBF16),
            "wkt": np.ascontiguousarray(wk[sl][perm].T).astype(NPBF16),
            "wvt": np.ascontiguousarray(wv[sl].T).astype(NPBF16),
            "wot": np.ascontiguousarray(wo[:, sl].T).astype(NPBF16),
            "cfull": cfull.astype(NPBF16),
            "sfull": sfull.astype(NPBF16),
            "tri": tri,
            "ones": ones,
            "eye2": eye2,
            "zs": zs,
        })
    return in_maps


def kernel(in_features, token_positions, wq, wk, wv, wo):
    _install_ntff_hook()
    in_maps = prepare_in_maps(dict(
        in_features=in_features, token_positions=token_positions,
        wq=wq, wk=wk, wv=wv, wo=wo))

    nc = _get_nc()
    res = run_bass_kernel_spmd(nc, in_maps, list(range(NCORES)))

    out = np.empty((B, S, D), dtype=np.float32)
    for b in range(B):
        acc = (np.asarray(res.results[2 * b]["outT"], dtype=np.float32)
               + np.asarray(res.results[2 * b + 1]["outT"], dtype=np.float32))
        out[b] = acc.T
    return out
